# revision 17
# baseline (speedup 1.0000x reference)
"""Causal self-attention (B=2, T=2048, D=1024, H=16) on 8 NeuronCores.

Sharding: heads across cores (2 heads/core). Each core:
  - loads x^T via hardware DMA-transpose (bf16), finer chunks first so
    compute can start early (all transposes on ONE HWDGE ring: concurrent
    transposes on the scalar ring corrupt data),
  - computes qT/kT/vT for its heads (W_qkv column shard), bf16 matmuls,
  - flash-style causal attention with scores transposed [k, q] so that
    att@v needs no transposes (ones-column on v gives softmax sums),
  - AllToAll (split in three, overlapped with compute) reshards y^T from
    head-sharded to row-sharded,
  - computes its 512-row slice of y @ W_proj with the full W_proj.
Order: A1(qkv t0-3) -> b0 attention (qs ascending) -> A2A#1 || A2(qkv t4-7)
       -> b1 attention (qs descending) with A2A#2 mid -> A2A#3 -> proj.
"""
import numpy as np
import ml_dtypes
from contextlib import ExitStack

import concourse.bass as bass
import concourse.tile as tile
from concourse import bacc, mybir
from concourse.bass_utils import run_bass_kernel_spmd
from concourse.masks import make_identity

B, T, D, H, HD = 2, 2048, 1024, 16, 64
NCORES = 8
BT = B * T                    # 4096
DQ = 128                      # head dims per core (2 heads x 64)
TT = 512                      # t-tile for the qkv phase
NTT = BT // TT                # 8
NDC = D // 128                # 8 contraction chunks
NQS = T // 512                # 4 q-supertiles per batch
SCALE = 1.0 / np.sqrt(HD)     # 0.125
ROWS = BT // NCORES           # 512 output rows per core
f32 = mybir.dt.float32
bf16 = mybir.dt.bfloat16
BF16NP = ml_dtypes.bfloat16


def build_module(debug=False):
    nc = bacc.Bacc("TRN2", target_bir_lowering=False, debug=False, num_devices=NCORES)
    x_d = nc.dram_tensor("x", [BT, D], bf16, kind="ExternalInput").ap()
    wqkv_d = nc.dram_tensor("wqkv", [D, 3, DQ], bf16, kind="ExternalInput").ap()
    wp_d = nc.dram_tensor("wp", [D, D], bf16, kind="ExternalInput").ap()
    out_d = nc.dram_tensor("out", [ROWS, D], f32, kind="ExternalOutput").ap()
    if debug:
        dbg = {
            "qT": nc.dram_tensor("dbg_qT", [128, BT], bf16, kind="ExternalOutput").ap(),
            "kT": nc.dram_tensor("dbg_kT", [128, BT], bf16, kind="ExternalOutput").ap(),
            "vext": nc.dram_tensor("dbg_vext", [128, BT // 128, 2, HD + 1], bf16, kind="ExternalOutput").ap(),
            "yTn": nc.dram_tensor("dbg_yTn", [128, BT], bf16, kind="ExternalOutput").ap(),
            "yall": nc.dram_tensor("dbg_yall", [128, NCORES, ROWS], bf16, kind="ExternalOutput").ap(),
            "xt0": nc.dram_tensor("dbg_xt0", [128, BT], bf16, kind="ExternalOutput").ap(),
        }

    with tile.TileContext(nc) as tc, ExitStack() as ctx:
        const = ctx.enter_context(tc.tile_pool(name="const", bufs=1))
        wpool = ctx.enter_context(tc.tile_pool(name="w", bufs=1))
        big = ctx.enter_context(tc.tile_pool(name="big", bufs=1))
        vtp = ctx.enter_context(tc.tile_pool(name="vt", bufs=2))
        expp = ctx.enter_context(tc.tile_pool(name="exp", bufs=3))
        smal = ctx.enter_context(tc.tile_pool(name="small", bufs=4))
        outp = ctx.enter_context(tc.tile_pool(name="outsb", bufs=2))
        psum = ctx.enter_context(tc.tile_pool(name="ps", bufs=1, space="PSUM"))
        dram = ctx.enter_context(tc.tile_pool(name="dram", bufs=1, space="DRAM"))

        ident = const.tile([128, 128], bf16)
        make_identity(nc, ident[:])
        zsb = const.tile([128, ROWS], bf16)
        nc.gpsimd.memset(zsb[:], 0.0)

        # Whole local W_qkv in SBUF: [d%128, d//128, {q,k,v}, dq]
        wqkv_sb = wpool.tile([128, NDC, 3, DQ], bf16)
        nc.gpsimd.dma_start(
            wqkv_sb[:], wqkv_d.rearrange("(c p) q m -> p c q m", p=128)
        )
        # Full W_proj in SBUF: [d%128, d//128, dout]
        wp_sb = wpool.tile([128, NDC, D], bf16)
        nc.gpsimd.dma_start(wp_sb[:], wp_d.rearrange("(c p) n -> p c n", p=128))

        # x^T resident in SBUF: per d-chunk, [128 d, BT] via HW DMA transpose
        xtf = []
        for dc in range(NDC):
            xt = big.tile([128, BT], bf16, tag=f"xtf{dc}", name=f"xtf{dc}")
            xtf.append(xt)
        for r0, r1 in ((0, 1024), (1024, 2048), (2048, 4096)):
            for dc in range(NDC):
                nc.sync.dma_start(
                    xtf[dc][:, r0:r1],
                    x_d[r0:r1, 128 * dc : 128 * (dc + 1)],
                    transpose=True,
                )

        qT_sb = big.tile([128, BT], bf16)    # [2 heads x 64, b*T+t]
        kT_sb = big.tile([128, BT], bf16)
        # v natural + ones column: [k%128, k//128, head, hd+1]
        vext_sb = big.tile([128, BT // 128, 2, HD + 1], bf16)
        nc.gpsimd.memset(vext_sb[:, :, :, HD : HD + 1], 1.0)
        yTn_sb = big.tile([128, BT], bf16)   # normalized y^T

        a2a_in = [None] * 3
        a2a_out = [None] * 3
        for i in range(3):
            a2a_in[i] = dram.tile([NCORES, 128, ROWS], bf16, name=f"a2a_in{i}")
            a2a_out[i] = dram.tile([NCORES, 128, ROWS], bf16, name=f"a2a_out{i}")
        # which collective carries which destination block j:
        #   C0: j=0..3 (batch 0), C1: j=7,6 (b1 qs3/qs2), C2: j=5,4
        cc_of_j = {0: 0, 1: 0, 2: 0, 3: 0, 7: 1, 6: 1, 5: 2, 4: 2}
        for i in range(3):
            for j in range(NCORES):
                if cc_of_j[j] != i:
                    nc.gpsimd.dma_start(a2a_in[i][j], zsb[:])

        def issue_cc(i):
            nc.gpsimd.collective_compute(
                "AllToAll",
                mybir.AluOpType.bypass,
                ins=[a2a_in[i].opt()],
                outs=[a2a_out[i].opt()],
                replica_groups=[list(range(NCORES))],
            )

        def qkv_tile(tt):
            # q -> k -> v sequentially: only 2 accumulator slots needed
            t0 = TT * tt
            outs = []
            for qi in range(3):
                ps = psum.tile([128, TT], f32, tag="acc", bufs=4, name=f"qkv{tt}_{qi}")
                for dc in range(NDC):
                    nc.tensor.matmul(
                        ps[:],
                        wqkv_sb[:, dc, qi, :],
                        xtf[dc][:, t0 : t0 + TT],
                        start=(dc == 0),
                        stop=(dc == NDC - 1),
                    )
                outs.append(ps)
                if qi == 0:
                    nc.vector.tensor_copy(qT_sb[:, t0 : t0 + TT], ps[:])
                elif qi == 1:
                    nc.vector.tensor_copy(kT_sb[:, t0 : t0 + TT], ps[:])
            vts = vtp.tile([128, TT], bf16, tag="vts", name=f"vts{tt}")
            nc.vector.tensor_copy(vts[:], outs[2][:])
            return vts

        def v_transposes(tt, vts):
            for s in range(4):
                vn_ps = psum.tile([128, 128], bf16, tag="tp", bufs=2, name=f"vn{tt}{s}")
                nc.tensor.transpose(vn_ps[:], vts[:, 128 * s : 128 * (s + 1)], ident[:])
                kchunk = 4 * tt + s
                nc.vector.tensor_copy(vext_sb[:, kchunk, 0, 0:HD], vn_ps[:, 0:HD])
                nc.vector.tensor_copy(vext_sb[:, kchunk, 1, 0:HD], vn_ps[:, HD:128])

        def emit_scores(b, qs, kt, qg):
            kg = 2048 * b + 128 * kt
            st_ps = psum.tile(
                [128, 2, 512], f32, tag="tp", bufs=2, name=f"st_{b}_{qs}_{kt}"
            )
            for h in range(2):
                nc.tensor.matmul(
                    st_ps[:, h, :],
                    kT_sb[64 * h : 64 * h + 64, kg : kg + 128],
                    qT_sb[64 * h : 64 * h + 64, qg : qg + 512],
                    start=True,
                    stop=True,
                )
            est = expp.tile([128, 2, 512], bf16, tag="est", name=f"est_{b}_{qs}_{kt}")
            nc.scalar.activation(
                est[:], st_ps[:], mybir.ActivationFunctionType.Exp, scale=SCALE
            )
            if kt >= 4 * qs:  # diagonal block: zero where k > q
                for h in range(2):
                    nc.gpsimd.affine_select(
                        out=est[:, h, :],
                        in_=est[:, h, :],
                        pattern=[[1, 512]],
                        compare_op=mybir.AluOpType.is_ge,
                        fill=0.0,
                        base=512 * qs - 128 * kt,
                        channel_multiplier=-1,
                    )
            return est

        def attention_block(b, qs, pre_emit=None):
            """One q-supertile of causal attention, software-pipelined so the
            PE never waits on the exp/mask chain."""
            qg = 2048 * b + 512 * qs
            nkt = 4 * qs + 4
            yt_ps0 = psum.tile([HD + 1, 512], f32, tag="acc", bufs=4, name=f"yt0_{b}_{qs}")
            yt_ps1 = psum.tile([HD + 1, 512], f32, tag="acc", bufs=4, name=f"yt1_{b}_{qs}")
            yt_ps = [yt_ps0, yt_ps1]
            est_next = emit_scores(b, qs, 0, qg)
            if pre_emit is not None:
                pre_emit()  # e.g. v-transposes for a tile, filling the PE
            for kt in range(nkt):
                est = est_next
                if kt + 1 < nkt:
                    est_next = emit_scores(b, qs, kt + 1, qg)
                kchunk = (2048 * b + 128 * kt) // 128
                for h in range(2):
                    nc.tensor.matmul(
                        yt_ps[h][:],
                        vext_sb[:, kchunk, h, :],
                        est[:, h, :],
                        start=(kt == 0),
                        stop=(kt == nkt - 1),
                    )
            # softmax normalization: sums live in row HD of each yt psum.
            su = smal.tile([33, 512], f32, tag="su", name=f"su_{b}_{qs}")
            nc.scalar.copy(su[0:1, :], yt_ps0[HD : HD + 1, :])
            nc.scalar.copy(su[32:33, :], yt_ps1[HD : HD + 1, :])
            rec = smal.tile([33, 512], f32, tag="rec", name=f"rec_{b}_{qs}")
            nc.vector.reciprocal(rec[:], su[:])
            for h in range(2):
                recb = smal.tile([1, 512], bf16, tag="recb", name=f"recb_{b}_{qs}_{h}")
                nc.vector.tensor_copy(recb[:], rec[32 * h : 32 * h + 1, :])
                bc = smal.tile([HD, 512], bf16, tag="bc", name=f"bc_{b}_{qs}_{h}")
                nc.gpsimd.partition_broadcast(bc[:], recb[:])
                nc.vector.tensor_mul(
                    yTn_sb[64 * h : 64 * h + 64, qg : qg + 512],
                    yt_ps[h][0:HD, :],
                    bc[:],
                )
            j = 4 * b + qs
            nc.sync.dma_start(
                a2a_in[cc_of_j[j]][j], yTn_sb[:, 512 * j : 512 * (j + 1)]
            )

        # ---- batch 0: qkv tiles 0..3, attention ascending, then A2A #0 ----
        vts_list = {}
        for tt in range(4):
            vts_list[tt] = qkv_tile(tt)
        attention_block(0, 0, pre_emit=(lambda: v_transposes(0, vts_list[0])))
        attention_block(0, 1, pre_emit=(lambda: v_transposes(1, vts_list[1])))
        attention_block(0, 2, pre_emit=(lambda: v_transposes(2, vts_list[2])))
        attention_block(0, 3, pre_emit=(lambda: v_transposes(3, vts_list[3])))
        issue_cc(0)
        # ---- batch 1: qkv tiles 4..7, attention descending ----
        for tt in range(4, NTT):
            vts_list[tt] = qkv_tile(tt)
        for tt in range(4, NTT):
            v_transposes(tt, vts_list[tt])
        attention_block(1, 3)
        attention_block(1, 2)
        issue_cc(1)  # carries j=7,6
        attention_block(1, 1)
        attention_block(1, 0)
        issue_cc(2)  # carries j=5,4

        # ---------------- projection ----------------
        ya = []
        for i in range(3):
            y = big.tile([128, NCORES, ROWS], bf16, name=f"ya{i}")
            for s in range(NCORES):
                nc.sync.dma_start(y[:, s, :], a2a_out[i][s])
            ya.append(y)
        yall_sb = big.tile([128, NCORES, ROWS], bf16)
        nc.vector.tensor_add(yall_sb[:], ya[0][:], ya[1][:])
        nc.vector.tensor_add(yall_sb[:], yall_sb[:], ya[2][:])
        for ts in range(ROWS // 128):
            ops = psum.tile([128, D], f32, tag="tp", bufs=2, name=f"ops{ts}")
            for s in range(NCORES):
                st = s == 0
                sp = s == NCORES - 1
                nc.tensor.matmul(
                    ops[:, 0:512],
                    yall_sb[:, s, 128 * ts : 128 * (ts + 1)],
                    wp_sb[:, s, 0:512],
                    start=st,
                    stop=sp,
                )
                nc.tensor.matmul(
                    ops[:, 512:1024],
                    yall_sb[:, s, 128 * ts : 128 * (ts + 1)],
                    wp_sb[:, s, 512:1024],
                    start=st,
                    stop=sp,
                )
            osb = outp.tile([128, D], f32, tag="osb", name=f"osb{ts}")
            nc.vector.tensor_copy(osb[:], ops[:])
            nc.sync.dma_start(out_d[128 * ts : 128 * (ts + 1), :], osb[:])

        if debug:
            nc.sync.dma_start(dbg["qT"][:], qT_sb[:])
            nc.sync.dma_start(dbg["kT"][:], kT_sb[:])
            nc.sync.dma_start(dbg["vext"][:], vext_sb[:])
            nc.sync.dma_start(dbg["yTn"][:], yTn_sb[:])
            nc.sync.dma_start(dbg["yall"][:], yall_sb[:])
            nc.sync.dma_start(dbg["xt0"][:], xtf[1][:])

    nc.compile()
    return nc


_NC_CACHE = None


def _get_module():
    global _NC_CACHE
    if _NC_CACHE is None:
        _NC_CACHE = build_module()
    return _NC_CACHE


def make_in_maps(x, W_qkv, W_proj):
    x2 = np.ascontiguousarray(
        np.asarray(x, dtype=np.float32).reshape(BT, D).astype(BF16NP)
    )
    wq = np.asarray(W_qkv, dtype=np.float32)
    wp = np.ascontiguousarray(np.asarray(W_proj, dtype=np.float32).astype(BF16NP))
    in_maps = []
    for c in range(NCORES):
        wl = np.ascontiguousarray(
            np.stack(
                [
                    wq[:, 128 * c : 128 * (c + 1)],
                    wq[:, D + 128 * c : D + 128 * (c + 1)],
                    wq[:, 2 * D + 128 * c : 2 * D + 128 * (c + 1)],
                ],
                axis=1,
            ).astype(BF16NP)
        )
        in_maps.append({"x": x2, "wqkv": wl, "wp": wp})
    return in_maps


def run(x, W_qkv, W_proj, **spmd_kwargs):
    nc = _get_module()
    in_maps = make_in_maps(x, W_qkv, W_proj)
    res = run_bass_kernel_spmd(nc, in_maps, list(range(NCORES)), **spmd_kwargs)
    out = np.concatenate([res.results[c]["out"] for c in range(NCORES)], axis=0)
    return out.reshape(B, T, D), res


def kernel(x, W_qkv, W_proj):
    out, _ = run(x, W_qkv, W_proj)
    return out


# revision 19
# speedup vs baseline: 1.0576x; 1.0576x over previous
"""Causal self-attention (B=2, T=2048, D=1024, H=16) on 8 NeuronCores.

Sharding: heads across cores (2 heads/core). Each core:
  - loads x^T via hardware DMA-transpose (bf16), finer chunks first so
    compute can start early (all transposes on ONE HWDGE ring: concurrent
    transposes on the scalar ring corrupt data),
  - computes qT/kT/vT for its heads (W_qkv column shard), bf16 matmuls,
  - flash-style causal attention with scores transposed [k, q] so that
    att@v needs no transposes (ones-column on v gives softmax sums),
  - AllToAll (split in three, overlapped with compute) reshards y^T from
    head-sharded to row-sharded,
  - computes its 512-row slice of y @ W_proj with the full W_proj.
Order: A1(qkv t0-3) -> b0 attention (qs ascending) -> A2A#1 || A2(qkv t4-7)
       -> b1 attention (qs descending) with A2A#2 mid -> A2A#3 -> proj.
"""
import numpy as np
import ml_dtypes
from contextlib import ExitStack

import concourse.bass as bass
import concourse.tile as tile
from concourse import bacc, mybir
from concourse.bass_utils import run_bass_kernel_spmd
from concourse.masks import make_identity

B, T, D, H, HD = 2, 2048, 1024, 16, 64
NCORES = 8
BT = B * T                    # 4096
DQ = 128                      # head dims per core (2 heads x 64)
TT = 512                      # t-tile for the qkv phase
NTT = BT // TT                # 8
NDC = D // 128                # 8 contraction chunks
NQS = T // 512                # 4 q-supertiles per batch
SCALE = 1.0 / np.sqrt(HD)     # 0.125
ROWS = BT // NCORES           # 512 output rows per core
f32 = mybir.dt.float32
bf16 = mybir.dt.bfloat16
BF16NP = ml_dtypes.bfloat16


def build_module(debug=False):
    nc = bacc.Bacc("TRN2", target_bir_lowering=False, debug=False, num_devices=NCORES)
    x_d = nc.dram_tensor("x", [BT, D], bf16, kind="ExternalInput").ap()
    wqkv_d = nc.dram_tensor("wqkv", [D, 3, DQ], bf16, kind="ExternalInput").ap()
    wp_d = nc.dram_tensor("wp", [D, D], bf16, kind="ExternalInput").ap()
    out_d = nc.dram_tensor("out", [ROWS, D], f32, kind="ExternalOutput").ap()
    if debug:
        dbg = {
            "qT": nc.dram_tensor("dbg_qT", [128, BT], bf16, kind="ExternalOutput").ap(),
            "kT": nc.dram_tensor("dbg_kT", [128, BT], bf16, kind="ExternalOutput").ap(),
            "vext": nc.dram_tensor("dbg_vext", [128, BT // 128, 2, HD + 1], bf16, kind="ExternalOutput").ap(),
            "yTn": nc.dram_tensor("dbg_yTn", [128, BT], bf16, kind="ExternalOutput").ap(),
            "yall": nc.dram_tensor("dbg_yall", [128, NCORES, ROWS], bf16, kind="ExternalOutput").ap(),
            "xt0": nc.dram_tensor("dbg_xt0", [128, BT], bf16, kind="ExternalOutput").ap(),
        }

    with tile.TileContext(nc) as tc, ExitStack() as ctx:
        const = ctx.enter_context(tc.tile_pool(name="const", bufs=1))
        wpool = ctx.enter_context(tc.tile_pool(name="w", bufs=1))
        big = ctx.enter_context(tc.tile_pool(name="big", bufs=1))
        vtp = ctx.enter_context(tc.tile_pool(name="vt", bufs=2))
        expp = ctx.enter_context(tc.tile_pool(name="exp", bufs=3))
        smal = ctx.enter_context(tc.tile_pool(name="small", bufs=4))
        outp = ctx.enter_context(tc.tile_pool(name="outsb", bufs=2))
        psum = ctx.enter_context(tc.tile_pool(name="ps", bufs=1, space="PSUM"))
        dram = ctx.enter_context(tc.tile_pool(name="dram", bufs=1, space="DRAM"))

        ident = const.tile([128, 128], bf16)
        make_identity(nc, ident[:])
        zsb = const.tile([128, ROWS], bf16)
        nc.gpsimd.memset(zsb[:], 0.0)

        # Whole local W_qkv in SBUF: [d%128, d//128, {q,k,v}, dq]
        wqkv_sb = wpool.tile([128, NDC, 3, DQ], bf16)
        nc.gpsimd.dma_start(
            wqkv_sb[:], wqkv_d.rearrange("(c p) q m -> p c q m", p=128)
        )
        # Full W_proj in SBUF: [d%128, d//128, dout]
        wp_sb = wpool.tile([128, NDC, D], bf16)
        nc.gpsimd.dma_start(wp_sb[:], wp_d.rearrange("(c p) n -> p c n", p=128))

        # x^T resident in SBUF: per d-chunk, [128 d, BT] via HW DMA transpose
        xtf = []
        for dc in range(NDC):
            xt = big.tile([128, BT], bf16, tag=f"xtf{dc}", name=f"xtf{dc}")
            xtf.append(xt)
        for r0, r1 in ((0, 1024), (1024, 2048), (2048, 4096)):
            for dc in range(NDC):
                nc.sync.dma_start(
                    xtf[dc][:, r0:r1],
                    x_d[r0:r1, 128 * dc : 128 * (dc + 1)],
                    transpose=True,
                )

        qT_sb = big.tile([128, BT], bf16)    # [2 heads x 64, b*T+t]
        kT_sb = big.tile([128, BT], bf16)
        # v natural + ones column: [k%128, k//128, head, hd+1]
        vext_sb = big.tile([128, BT // 128, 2, HD + 1], bf16)
        nc.gpsimd.memset(vext_sb[:, :, :, HD : HD + 1], 1.0)
        yTn_sb = big.tile([128, BT], bf16)   # normalized y^T

        a2a_in = [None] * 2
        a2a_out = [None] * 2
        for i in range(2):
            a2a_in[i] = dram.tile([NCORES, 128, ROWS], bf16, name=f"a2a_in{i}")
            a2a_out[i] = dram.tile([NCORES, 128, ROWS], bf16, name=f"a2a_out{i}")
        # C0 carries j=0..3 (batch 0), C1 carries j=4..7 (batch 1)
        cc_of_j = {j: (0 if j < 4 else 1) for j in range(NCORES)}
        for j in range(4):
            nc.gpsimd.dma_start(a2a_in[1][j], zsb[:])
            nc.gpsimd.dma_start(a2a_in[0][j + 4], zsb[:])

        def issue_cc(i):
            # issued from the SP queue: fire-and-forget; completion gates only
            # the yall loads emitted at the very end of the program.
            nc.gpsimd.collective_compute(
                "AllToAll",
                mybir.AluOpType.bypass,
                ins=[a2a_in[i].opt()],
                outs=[a2a_out[i].opt()],
                replica_groups=[list(range(NCORES))],
            )

        def qkv_tile(tt):
            # q -> k -> v sequentially: only 2 accumulator slots needed
            t0 = TT * tt
            outs = []
            for qi in range(3):
                ps = psum.tile([128, TT], f32, tag="acc", bufs=4, name=f"qkv{tt}_{qi}")
                for dc in range(NDC):
                    nc.tensor.matmul(
                        ps[:],
                        wqkv_sb[:, dc, qi, :],
                        xtf[dc][:, t0 : t0 + TT],
                        start=(dc == 0),
                        stop=(dc == NDC - 1),
                    )
                outs.append(ps)
                if qi == 0:
                    nc.vector.tensor_copy(qT_sb[:, t0 : t0 + TT], ps[:])
                elif qi == 1:
                    nc.vector.tensor_copy(kT_sb[:, t0 : t0 + TT], ps[:])
            vts = vtp.tile([128, TT], bf16, tag="vts", name=f"vts{tt}")
            nc.vector.tensor_copy(vts[:], outs[2][:])
            return vts

        def v_transposes(tt, vts):
            for s in range(4):
                vn_ps = psum.tile([128, 128], bf16, tag="tp", bufs=2, name=f"vn{tt}{s}")
                nc.tensor.transpose(vn_ps[:], vts[:, 128 * s : 128 * (s + 1)], ident[:])
                kchunk = 4 * tt + s
                nc.vector.tensor_copy(vext_sb[:, kchunk, 0, 0:HD], vn_ps[:, 0:HD])
                nc.vector.tensor_copy(vext_sb[:, kchunk, 1, 0:HD], vn_ps[:, HD:128])

        def emit_scores(b, qs, kt, qg):
            kg = 2048 * b + 128 * kt
            st_ps = psum.tile(
                [128, 2, 512], f32, tag="tp", bufs=2, name=f"st_{b}_{qs}_{kt}"
            )
            for h in range(2):
                nc.tensor.matmul(
                    st_ps[:, h, :],
                    kT_sb[64 * h : 64 * h + 64, kg : kg + 128],
                    qT_sb[64 * h : 64 * h + 64, qg : qg + 512],
                    start=True,
                    stop=True,
                )
            est = expp.tile([128, 2, 512], bf16, tag="est", name=f"est_{b}_{qs}_{kt}")
            nc.scalar.activation(
                est[:], st_ps[:], mybir.ActivationFunctionType.Exp, scale=SCALE
            )
            if kt >= 4 * qs:  # diagonal block: zero where k > q
                for h in range(2):
                    nc.gpsimd.affine_select(
                        out=est[:, h, :],
                        in_=est[:, h, :],
                        pattern=[[1, 512]],
                        compare_op=mybir.AluOpType.is_ge,
                        fill=0.0,
                        base=512 * qs - 128 * kt,
                        channel_multiplier=-1,
                    )
            return est

        def attention_block(b, qs, pre_emit=None):
            """One q-supertile of causal attention, software-pipelined so the
            PE never waits on the exp/mask chain."""
            qg = 2048 * b + 512 * qs
            nkt = 4 * qs + 4
            yt_ps0 = psum.tile([HD + 1, 512], f32, tag="acc", bufs=4, name=f"yt0_{b}_{qs}")
            yt_ps1 = psum.tile([HD + 1, 512], f32, tag="acc", bufs=4, name=f"yt1_{b}_{qs}")
            yt_ps = [yt_ps0, yt_ps1]
            est_next = emit_scores(b, qs, 0, qg)
            if pre_emit is not None:
                pre_emit()  # e.g. v-transposes for a tile, filling the PE
            for kt in range(nkt):
                est = est_next
                if kt + 1 < nkt:
                    est_next = emit_scores(b, qs, kt + 1, qg)
                kchunk = (2048 * b + 128 * kt) // 128
                for h in range(2):
                    nc.tensor.matmul(
                        yt_ps[h][:],
                        vext_sb[:, kchunk, h, :],
                        est[:, h, :],
                        start=(kt == 0),
                        stop=(kt == nkt - 1),
                    )
            # softmax normalization: sums live in row HD of each yt psum.
            su = smal.tile([33, 512], f32, tag="su", name=f"su_{b}_{qs}")
            nc.scalar.copy(su[0:1, :], yt_ps0[HD : HD + 1, :])
            nc.scalar.copy(su[32:33, :], yt_ps1[HD : HD + 1, :])
            rec = smal.tile([33, 512], f32, tag="rec", name=f"rec_{b}_{qs}")
            nc.vector.reciprocal(rec[:], su[:])
            for h in range(2):
                recb = smal.tile([1, 512], bf16, tag="recb", name=f"recb_{b}_{qs}_{h}")
                nc.vector.tensor_copy(recb[:], rec[32 * h : 32 * h + 1, :])
                bc = smal.tile([HD, 512], bf16, tag="bc", name=f"bc_{b}_{qs}_{h}")
                nc.gpsimd.partition_broadcast(bc[:], recb[:])
                nc.vector.tensor_mul(
                    yTn_sb[64 * h : 64 * h + 64, qg : qg + 512],
                    yt_ps[h][0:HD, :],
                    bc[:],
                )
            j = 4 * b + qs
            nc.sync.dma_start(
                a2a_in[cc_of_j[j]][j], yTn_sb[:, 512 * j : 512 * (j + 1)]
            )

        # ---- batch 0: qkv tiles 0..3, attention ascending, then A2A #0 ----
        vts_list = {}
        for tt in range(4):
            vts_list[tt] = qkv_tile(tt)
        attention_block(0, 0, pre_emit=(lambda: v_transposes(0, vts_list[0])))
        attention_block(0, 1, pre_emit=(lambda: v_transposes(1, vts_list[1])))
        attention_block(0, 2, pre_emit=(lambda: v_transposes(2, vts_list[2])))
        attention_block(0, 3, pre_emit=(lambda: v_transposes(3, vts_list[3])))
        issue_cc(0)
        # ---- batch 1: qkv tiles 4..7, attention descending ----
        for tt in range(4, NTT):
            vts_list[tt] = qkv_tile(tt)
        attention_block(1, 0, pre_emit=(lambda: v_transposes(4, vts_list[4])))
        attention_block(1, 1, pre_emit=(lambda: v_transposes(5, vts_list[5])))
        attention_block(1, 2, pre_emit=(lambda: v_transposes(6, vts_list[6])))
        attention_block(1, 3, pre_emit=(lambda: v_transposes(7, vts_list[7])))
        issue_cc(1)  # carries j=4..7

        # ---------------- projection ----------------
        ya = []
        for i in range(2):
            y = big.tile([128, NCORES, ROWS], bf16, name=f"ya{i}")
            for s in range(NCORES):
                nc.sync.dma_start(y[:, s, :], a2a_out[i][s])
            ya.append(y)
        yall_sb = big.tile([128, NCORES, ROWS], bf16)
        nc.vector.tensor_add(yall_sb[:], ya[0][:], ya[1][:])
        for ts in range(ROWS // 128):
            ops = psum.tile([128, D], f32, tag="tp", bufs=2, name=f"ops{ts}")
            for s in range(NCORES):
                st = s == 0
                sp = s == NCORES - 1
                nc.tensor.matmul(
                    ops[:, 0:512],
                    yall_sb[:, s, 128 * ts : 128 * (ts + 1)],
                    wp_sb[:, s, 0:512],
                    start=st,
                    stop=sp,
                )
                nc.tensor.matmul(
                    ops[:, 512:1024],
                    yall_sb[:, s, 128 * ts : 128 * (ts + 1)],
                    wp_sb[:, s, 512:1024],
                    start=st,
                    stop=sp,
                )
            osb = outp.tile([128, D], f32, tag="osb", name=f"osb{ts}")
            nc.vector.tensor_copy(osb[:], ops[:])
            nc.sync.dma_start(out_d[128 * ts : 128 * (ts + 1), :], osb[:])

        if debug:
            nc.sync.dma_start(dbg["qT"][:], qT_sb[:])
            nc.sync.dma_start(dbg["kT"][:], kT_sb[:])
            nc.sync.dma_start(dbg["vext"][:], vext_sb[:])
            nc.sync.dma_start(dbg["yTn"][:], yTn_sb[:])
            nc.sync.dma_start(dbg["yall"][:], yall_sb[:])
            nc.sync.dma_start(dbg["xt0"][:], xtf[1][:])

    nc.compile()
    return nc


_NC_CACHE = None


def _get_module():
    global _NC_CACHE
    if _NC_CACHE is None:
        _NC_CACHE = build_module()
    return _NC_CACHE


def make_in_maps(x, W_qkv, W_proj):
    x2 = np.ascontiguousarray(
        np.asarray(x, dtype=np.float32).reshape(BT, D).astype(BF16NP)
    )
    wq = np.asarray(W_qkv, dtype=np.float32)
    wp = np.ascontiguousarray(np.asarray(W_proj, dtype=np.float32).astype(BF16NP))
    in_maps = []
    for c in range(NCORES):
        wl = np.ascontiguousarray(
            np.stack(
                [
                    wq[:, 128 * c : 128 * (c + 1)],
                    wq[:, D + 128 * c : D + 128 * (c + 1)],
                    wq[:, 2 * D + 128 * c : 2 * D + 128 * (c + 1)],
                ],
                axis=1,
            ).astype(BF16NP)
        )
        in_maps.append({"x": x2, "wqkv": wl, "wp": wp})
    return in_maps


def run(x, W_qkv, W_proj, **spmd_kwargs):
    nc = _get_module()
    in_maps = make_in_maps(x, W_qkv, W_proj)
    res = run_bass_kernel_spmd(nc, in_maps, list(range(NCORES)), **spmd_kwargs)
    out = np.concatenate([res.results[c]["out"] for c in range(NCORES)], axis=0)
    return out.reshape(B, T, D), res


def kernel(x, W_qkv, W_proj):
    out, _ = run(x, W_qkv, W_proj)
    return out
